# revision 1
# baseline (speedup 1.0000x reference)
"""MoE top-2 expert projection kernel for 8 Trainium2 NeuronCores — v2.

Per the sharding hint ("token dispatch by top-k index"), the host computes
the top-2 routing *decision* and uses it only to build the dispatch layout:
tokens are balanced across the 8 cores so each core's per-expert slot counts
fit a uniform per-expert capacity (multiple of 128, same on every core =>
one SPMD program), and the dispatched activations are staged pre-gathered /
transposed in fp16.  All of the reference arithmetic — gate logits, softmax
values, expert projections, bias, gate scaling and the top-2 combine — runs
on the NeuronCores:

  per slot-tile (128 slots of one expert e):
    psum_g = xd^T Wg            fp16 matmuls, fp32 psum   (gate logits)
    psum_y = xd^T W_e           fp16 matmuls, fp32 psum
    g      = softmax(psum_g + bg)[:, e]                   (DVE + ACT exp)
    y16    = (psum_y + bcast(b_e)) * g                    (DVE add, ACT scale)
    scatter y16 rows -> yk[2*token + rank]                (indirect DMA)
  then per token-tile: out[t] = yk[2t] + yk[2t+1]         (DVE add)

The scatter uses plain writes into disjoint rows (each token has exactly one
rank-1 and one rank-2 row), so there are no read-modify-write races; padding
slots land in a dump row.  Compute is top-2 sparse: ~19 slot-tiles/core vs
64 tile-equivalents for the dense all-expert baseline.
"""

import sys

if "/opt/trn_rl_repo" not in sys.path:
    sys.path.insert(0, "/opt/trn_rl_repo")

import numpy as np

import concourse.bass as bass
import concourse.mybir as mybir
import concourse.tile as tile
from concourse import bacc
from concourse.bass import ts
from concourse.bass_utils import run_bass_kernel_spmd
from concourse.masks import make_identity

# Problem sizes (hardcoded per the harness contract).
B, S, D, O, E = 4, 2048, 1024, 1024, 8
TOK = B * S                  # 8192 tokens total
N_CORES = 8
TPC = TOK // N_CORES         # 1024 tokens per core
P = 128                      # SBUF partitions
KO = D // P                  # 8 contraction blocks
NH = O // 512                # PSUM halves of the output dim
DUMP = 2 * TPC               # yk dump row for padding slots
EL = 5                       # experts resident per core (5-of-8 cover)
# core c owns the 5 experts NOT in the cyclic complement {c, c+1, c+3}
COMPS = [set(((0 + i) % 8, (1 + i) % 8, (3 + i) % 8)) for i in range(N_CORES)]
SUBS = [sorted(set(range(E)) - c) for c in COMPS]

F16 = mybir.dt.float16
F32 = mybir.dt.float32
I32 = mybir.dt.int32


def build_nc(ce, rows, thr):
    """ce: per-expert slot capacity per core (multiples of 128).
    rows: per-expert scatter row count (>= max real count over cores).
    thr: per readback-half, the max expert index any of its tokens uses;
    the half's readback fires once experts <= thr[h] have scattered."""
    ce = tuple(int(c) for c in ce)
    rows = tuple(int(r) for r in rows)
    thr = tuple(int(t) for t in thr)
    S_slots = sum(ce)
    NT = S_slots // P
    # tile -> local-expert map and per-local-expert first-tile flag
    tile_expert = []
    for e in range(EL):
        tile_expert += [e] * (ce[e] // P)
    tile0 = [0] * EL
    start = 0
    for e in range(EL):
        tile0[e] = start
        start += ce[e] // P

    nc = bacc.Bacc(None, target_bir_lowering=False)

    # xd quarters staged pre-shuffled: xdq[q][p, ko*qS+s] so each DMA
    # moves 128 fully-contiguous lines (full DMA efficiency)
    NQ = 4
    qtiles = [(S_slots // P) * (q + 1) // NQ - (S_slots // P) * q // NQ
              for q in range(NQ)]
    qstart = [sum(qtiles[:q]) for q in range(NQ)]
    xdq = [
        nc.dram_tensor(f"xd{q}", [P, KO * qtiles[q] * P], F16,
                       kind="ExternalInput")
        for q in range(NQ)
    ]
    WT = nc.dram_tensor("WT", [EL, P, KO * O], F16, kind="ExternalInput")
    WgT = nc.dram_tensor("WgT", [P, KO * E], F16, kind="ExternalInput")
    b_in = nc.dram_tensor("b", [1, EL * O], F16, kind="ExternalInput")
    bg_in = nc.dram_tensor("bg", [1, E], F16, kind="ExternalInput")
    idx_in = nc.dram_tensor("idx", [P, NT], I32, kind="ExternalInput")
    yk = nc.dram_tensor("yk", [2 * TPC + 8, O], F16, kind="Internal")
    # out rows live as out[p, tt*O:(tt+1)*O] = token tt*128+p (host unmaps)
    out = nc.dram_tensor("out", [P, (TPC // P) * O], F16, kind="ExternalOutput")

    scat_sem = nc.alloc_semaphore("scat_sem")
    n_scat = [0]
    scat_after = [0] * EL

    with tile.TileContext(nc) as tc:
        with (
            tc.tile_pool(name="resident", bufs=1) as rpool,
            tc.tile_pool(name="work", bufs=3) as wpool,
            tc.tile_pool(name="bias", bufs=2) as bpool,
            tc.tile_pool(name="rb", bufs=2) as rbpool,
            tc.tile_pool(name="ob", bufs=4) as opool,
            tc.tile_pool(name="wcyc", bufs=2) as wcpool,
            tc.tile_pool(name="psum_y", bufs=3, space="PSUM") as ypool,
            tc.tile_pool(name="psum_g", bufs=2, space="PSUM") as gpool,
        ):
            xd16 = rpool.tile([P, KO, S_slots], F16, tag="xd16")
            wg16 = rpool.tile([P, KO, E], F16, tag="wg16")
            b16 = rpool.tile([1, EL * O], F16, tag="b16")
            bg16 = rpool.tile([1, E], F16, tag="bg16")
            bgb32 = rpool.tile([P, E], F32, tag="bgb32")
            idx32 = rpool.tile([P, NT], I32, tag="idx32")
            ones16 = rpool.tile([1, P], F16, tag="ones16")
            ident = rpool.tile([P, P], F32, tag="ident")
            w_bufs = [
                wcpool.tile([P, KO, O], F16, tag="wcyc", name=f"wbuf{i}")
                for i in range(2)
            ]
            bb_bufs = [
                bpool.tile([P, O], F32, tag="bb", name=f"bb{i}") for i in range(2)
            ]

            def load_x_quarter(q):
                if qtiles[q] == 0:
                    return
                qsl = slice(qstart[q] * P, (qstart[q] + qtiles[q]) * P)
                nc.sync.dma_start(
                    xd16[:, :, qsl],
                    xdq[q][:, :].rearrange("p (ko t) -> p ko t", ko=KO),
                )

            def load_w(e, eng=None):
                (eng or nc.sync).dma_start(
                    w_bufs[e % 2][:],
                    WT[e].rearrange("p (ko o) -> p ko o", ko=KO),
                )

            # ---- initial loads + PE warmup ------------------------------
            nc.gpsimd.memset(ones16[:], 1.0)
            make_identity(nc, ident[:])
            load_x_quarter(0)
            load_w(0, nc.scalar)
            nc.sync.dma_start(
                wg16[:], WgT[:, :].rearrange("p (ko e) -> p ko e", ko=KO)
            )
            load_x_quarter(1)
            nc.sync.dma_start(bg16[:], bg_in[:, :])
            nc.sync.dma_start(b16[:], b_in[:, :])
            nc.sync.dma_start(idx32[:], idx_in[:, :])
            for q in range(2, NQ):
                load_x_quarter(q)

            # 128-col fp32 warmups: ~3us of continuous PE work ramps the
            # tensor engine to its full p-state before the real tiles start
            for i in range(14):
                warm_ps = ypool.tile([P, O], F32, tag="y", name=f"warm{i}")
                nc.tensor.matmul(
                    warm_ps[:, 0:P], lhsT=ident[:], rhs=ident[:],
                    start=True, stop=True,
                )
            # bg broadcast to all partitions (once)
            psum_bg = gpool.tile([P, E], F32, tag="g", name="bgb")
            nc.tensor.matmul(
                psum_bg[:], lhsT=ones16[:], rhs=bg16[:], start=True, stop=True
            )
            nc.any.tensor_copy(bgb32[:], psum_bg[:])

            # ---- slot tiles ---------------------------------------------
            for j in range(NT):
                e = tile_expert[j]
                tsl = ts(j, P)
                if j == tile0[e]:
                    # bias row of expert e broadcast to 128 partitions
                    psum_b = ypool.tile([P, O], F32, tag="y", name=f"pb{e}")
                    for h in range(NH):
                        hsl = ts(h, 512)
                        nc.tensor.matmul(
                            psum_b[:, hsl], lhsT=ones16[:],
                            rhs=b16[0:1, e * O + h * 512:e * O + (h + 1) * 512],
                            start=True, stop=True,
                        )
                    nc.any.tensor_copy(bb_bufs[e % 2][:], psum_b[:])
                if j == tile0[e] + ce[e] // P - 1 and e + 1 < EL:
                    # last tile of expert e: enqueue W[e+1]; the DMA waits on
                    # the buffer release so an early issue only hides latency
                    load_w(e + 1)

                xsrc, xsl = xd16, tsl
                # gate logits for this tile
                psum_g = gpool.tile([P, E], F32, tag="g")
                for ko in range(KO):
                    nc.tensor.matmul(
                        psum_g[:], lhsT=xsrc[:, ko, xsl], rhs=wg16[:, ko, :],
                        start=(ko == 0), stop=(ko == KO - 1),
                    )
                # expert matmuls
                psum_y = ypool.tile([P, O], F32, tag="y")
                for ko in range(KO):
                    for h in range(NH):
                        hsl = ts(h, 512)
                        nc.tensor.matmul(
                            psum_y[:, hsl],
                            lhsT=xsrc[:, ko, xsl],
                            rhs=w_bufs[e % 2][:, ko, hsl],
                            start=(ko == 0),
                            stop=(ko == KO - 1),
                        )

                # softmax value of expert e (gate weight)
                logits = wpool.tile([P, E], F32, tag="lg")
                nc.vector.tensor_tensor(
                    logits[:], psum_g[:], bgb32[:], mybir.AluOpType.add
                )
                m1 = wpool.tile([P, 1], F32, tag="m1")
                nc.vector.tensor_reduce(
                    m1[:], logits[:], mybir.AxisListType.X, mybir.AluOpType.max
                )
                negm1 = wpool.tile([P, 1], F32, tag="nm")
                nc.vector.tensor_scalar_mul(negm1[:], m1[:], -1.0)
                ex = wpool.tile([P, E], F32, tag="ex")
                nc.scalar.activation(
                    ex[:], logits[:], mybir.ActivationFunctionType.Exp,
                    bias=negm1[:, 0:1], scale=1.0,
                )
                ssum = wpool.tile([P, 1], F32, tag="ss")
                nc.vector.tensor_reduce(
                    ssum[:], ex[:], mybir.AxisListType.X, mybir.AluOpType.add
                )
                rsum = wpool.tile([P, 1], F32, tag="rs")
                nc.vector.reciprocal(rsum[:], ssum[:])
                g_col = wpool.tile([P, 1], F32, tag="gc")
                nc.vector.tensor_tensor(
                    g_col[:], ex[:, e:e + 1], rsum[:], mybir.AluOpType.mult
                )

                # y16 = (psum_y + b_bcast) * g ; scatter rows to yk
                y16a = wpool.tile([P, O], F16, tag="ya")
                nc.vector.tensor_tensor(
                    y16a[:], psum_y[:], bb_bufs[e % 2][:], mybir.AluOpType.add
                )
                y16 = wpool.tile([P, O], F16, tag="yb")
                nc.scalar.activation(
                    y16[:], y16a[:], mybir.ActivationFunctionType.Copy,
                    bias=0.0, scale=g_col[:, 0:1],
                )
                jrel = j - tile0[e]
                nrow = min(P, max(0, rows[e] - jrel * P))
                if nrow > 0:
                    nc.gpsimd.indirect_dma_start(
                        out=yk[:, :],
                        out_offset=bass.IndirectOffsetOnAxis(
                            ap=idx32[0:nrow, j:j + 1], axis=0
                        ),
                        in_=y16[0:nrow, :],
                        in_offset=None,
                    ).then_inc(scat_sem, 16)
                if nrow > 0:
                    n_scat[0] += 1
                    scat_after[e] = n_scat[0]

            # ---- combine: out[t] = yk[2t] + yk[2t+1] --------------------
            # explicit completion barrier: the scatters' DRAM writes are not
            # tracked as producers of the readback DMAs below.  All readbacks
            # issue back-to-back on the sync queue; the dependent out-stores
            # go through the vector engine's queue so they don't serialize
            # the readback stream.
            NRB = TPC // P
            # two big readbacks (16KB/partition contiguous lines), one per
            # queue; each covers half the tokens.  Tokens are sorted by
            # max-expert on the host, so half 0 only needs experts <= thr[0]
            # and its readback overlaps the remaining experts' compute.
            rbs = []
            for h in range(2):
                eng = nc.sync if h == 0 else nc.scalar
                eng.wait_ge(scat_sem, scat_after[thr[h]] * 16)
                rb = rbpool.tile([P, NRB, O], F16, tag="rb", name=f"rb{h}")
                eng.dma_start(
                    rb[:],
                    yk[h * TPC:(h + 1) * TPC, :].rearrange(
                        "(p r) c -> p r c", p=P
                    ),
                )
                rbs.append(rb)
            # rb[h][p, r, :]: yk row h*1024 + p*8 + r = token-pair row
            # (2t + rank) with t = (h*1024 + p*8 + r) // 2: partition p holds
            # tokens 4h*128... NOT token-tile aligned; define out layout to
            # match: out[p, i*O:(i+1)*O] = token (i) per map below (host unmaps)
            for h in range(2):
                for i in range(NRB // 2):
                    o16 = opool.tile([P, O], F16, tag="o16", name=f"o{h}_{i}")
                    nc.vector.tensor_tensor(
                        o16[:],
                        rbs[h][:, 2 * i, :], rbs[h][:, 2 * i + 1, :],
                        mybir.AluOpType.add,
                    )
                    eng = nc.scalar if h == 0 else nc.sync
                    eng.dma_start(
                        out[:, (h * (NRB // 2) + i) * O:
                               (h * (NRB // 2) + i + 1) * O],
                        o16[:],
                    )

    nc.compile()
    return nc


_NC_CACHE = {}


def _get_nc(ce, rows, thr):
    key = (tuple(ce), tuple(rows), tuple(thr))
    if key not in _NC_CACHE:
        _NC_CACHE[key] = build_nc(*key)
    return _NC_CACHE[key]


def _route(x2d, Wg, bg):
    """Host-side top-2 routing + 5-of-8 cover balancing.

    Each token goes to a core whose 5-expert subset contains both its
    experts.  Pair-level water-fill with a capacity-aware cost keeps the
    per-(core,expert) counts near CNT_e/5, then a repair pass fixes the
    per-core totals to exactly TPC."""
    logits = x2d.astype(np.float32) @ Wg.astype(np.float32).T + bg
    m = logits.max(axis=1, keepdims=True)
    p = np.exp(logits - m, dtype=np.float32)
    p /= p.sum(axis=1, keepdims=True)
    order = np.argsort(-p, axis=1, kind="stable")
    e1 = order[:, 0].astype(np.int64)
    e2 = order[:, 1].astype(np.int64)

    CNT = np.bincount(np.concatenate([e1, e2]), minlength=E)
    T = CNT / 5.0
    pairs = {}
    for t in range(TOK):
        k = (min(e1[t], e2[t]), max(e1[t], e2[t]))
        pairs.setdefault(k, []).append(t)
    cand = {k: [c for c in range(N_CORES)
                if k[0] in SUBS[c] and k[1] in SUBS[c]] for k in pairs}
    assert all(cand[k] for k in pairs)
    cnt = np.zeros((N_CORES, E))
    tot = np.zeros(N_CORES, np.int64)
    quota = {k: {c: 0 for c in cand[k]} for k in pairs}
    porder = sorted(pairs, key=lambda k: (len(cand[k]), -len(pairs[k])))
    rem = {k: len(pairs[k]) for k in pairs}
    while any(rem.values()):
        for k in porder:
            if rem[k] == 0:
                continue
            a, b = k
            cs = [c for c in cand[k] if tot[c] < TPC] or cand[k]
            c = min(cs, key=lambda c: (max(cnt[c, a] - T[a],
                                           cnt[c, b] - T[b]), tot[c]))
            quota[k][c] += 1
            cnt[c, a] += 1
            cnt[c, b] += 1
            tot[c] += 1
            rem[k] -= 1
    for _ in range(20000):
        hi, lo = int(np.argmax(tot)), int(np.argmin(tot))
        if tot[hi] <= TPC:
            break
        best = None
        for k in pairs:
            if hi in cand[k] and lo in cand[k] and quota[k][hi] > 0:
                a, b = k
                gain = (max(cnt[hi, a] - T[a], cnt[hi, b] - T[b])
                        - max(cnt[lo, a] - T[a] + 1, cnt[lo, b] - T[b] + 1))
                if best is None or gain > best[0]:
                    best = (gain, k)
        assert best is not None, "cover repair stuck"
        k = best[1]
        a, b = k
        quota[k][hi] -= 1
        quota[k][lo] += 1
        cnt[hi, a] -= 1
        cnt[hi, b] -= 1
        cnt[lo, a] += 1
        cnt[lo, b] += 1
        tot[hi] -= 1
        tot[lo] += 1
    assert (tot == TPC).all()
    # token -> core from quotas (tokens of a pair are interchangeable)
    core = np.full(TOK, -1, np.int64)
    for k, q in quota.items():
        ts_ = pairs[k]
        i = 0
        for c, n in q.items():
            core[ts_[i:i + n]] = c
            i += n
        assert i == len(ts_)
    cnti = cnt.astype(np.int64)
    return e1, e2, core, cnti


def _prepare(inputs):
    x = np.ascontiguousarray(np.asarray(inputs["x"], np.float32)).reshape(TOK, D)
    W = np.asarray(inputs["W"], np.float32)
    bb = np.asarray(inputs["b"], np.float32)
    Wg = np.asarray(inputs["Wg"], np.float32)
    bg = np.asarray(inputs["bg"], np.float32)

    e1, e2, core, cnt = _route(x, Wg, bg)
    # locals per core ordered by count desc -> uniform caps by rank position
    locs = [sorted(SUBS[c], key=lambda e: -cnt[c, e]) for c in range(N_CORES)]
    lcnt = np.array([[cnt[c, locs[c][l]] for l in range(EL)]
                     for c in range(N_CORES)])
    ce = tuple(int(128 * -(-int(lcnt[:, l].max()) // 128)) for l in range(EL))
    rows = tuple(int(min(ce[l], -(-int(lcnt[:, l].max()) // 8) * 8))
                 for l in range(EL))
    S_slots = sum(ce)
    NT = S_slots // P
    estart = np.concatenate([[0], np.cumsum(ce)])

    WgTf = Wg.T.astype(np.float32)          # [D, E]
    in_maps = []
    tok_lists = []
    maxl_all = []
    for c in range(N_CORES):
        loc = locs[c]
        loc_of = {e: l for l, e in enumerate(loc)}
        perm = loc + sorted(set(range(E)) - set(loc))
        toks = np.nonzero(core == c)[0]
        assert toks.size == TPC
        # sort by max LOCAL index so early yk rows depend on early tiles
        maxl = np.array([max(loc_of[int(e1[t])], loc_of[int(e2[t])])
                         for t in toks])
        o_ = np.argsort(maxl, kind="stable")
        toks = toks[o_]
        maxl = maxl[o_]
        maxl_all.append((int(maxl[:TPC // 2].max()), int(maxl.max())))
        local = np.full(TOK, -1, np.int64)
        local[toks] = np.arange(TPC)

        xd = np.zeros((S_slots, D), np.float16)
        idx = np.full(S_slots, DUMP, np.int32)
        for l in range(EL):
            e = loc[l]
            t1 = toks[e1[toks] == e]
            t2 = toks[e2[toks] == e]
            srcs = np.concatenate([t1, t2])
            dst = np.concatenate([2 * local[t1], 2 * local[t2] + 1])
            assert srcs.size <= ce[l], (c, l, srcs.size, ce[l])
            s0 = estart[l]
            xd[s0:s0 + srcs.size] = x[srcs].astype(np.float16)
            idx[s0:s0 + srcs.size] = dst
        im = {}
        # W for the 5 local experts: [EL, P, KO*O]
        im["WT"] = np.ascontiguousarray(
            W[loc].reshape(EL, O, KO, P).transpose(0, 3, 2, 1)
            .reshape(EL, P, KO * O).astype(np.float16)
        )
        # Wg columns permuted so local expert l is logits column l
        im["WgT"] = np.ascontiguousarray(
            WgTf[:, perm].reshape(KO, P, E).transpose(1, 0, 2)
            .reshape(P, KO * E).astype(np.float16)
        )
        im["bg"] = np.ascontiguousarray(
            bg[perm].reshape(1, E).astype(np.float16))
        im["b"] = np.ascontiguousarray(
            bb[loc].reshape(1, EL * O).astype(np.float16))
        xdT = np.ascontiguousarray(xd.T)
        NQ = 4
        qtiles = [NT * (q + 1) // NQ - NT * q // NQ for q in range(NQ)]
        qstart = [sum(qtiles[:q]) for q in range(NQ)]
        for q in range(NQ):
            qsl = slice(qstart[q] * P, (qstart[q] + qtiles[q]) * P)
            im[f"xd{q}"] = np.ascontiguousarray(
                xdT[:, qsl].reshape(KO, P, qtiles[q] * P)
                .transpose(1, 0, 2).reshape(P, KO * qtiles[q] * P)
            )
        im["idx"] = np.ascontiguousarray(idx.reshape(NT, P).T.astype(np.int32))
        in_maps.append(im)
        tok_lists.append(toks)
    thr = (max(h[0] for h in maxl_all), max(h[1] for h in maxl_all))
    return ce, rows, thr, in_maps, tok_lists


def run(inputs, **spmd_kwargs):
    ce, rows, thr, in_maps, tok_lists = _prepare(inputs)
    nc = _get_nc(ce, rows, thr)
    res = None
    for attempt in range(3):
        try:
            res = run_bass_kernel_spmd(
                nc, in_maps, core_ids=list(range(N_CORES)), **spmd_kwargs
            )
            break
        except Exception:
            if attempt == 2:
                raise
    out = np.empty((TOK, O), np.float32)
    # device out layout [P, 8*O]: slot s = h*4+i holds token h*512 + 4*p + i
    pp, ss = np.meshgrid(np.arange(P), np.arange(8), indexing="ij")
    tl = ((ss // 4) * (TPC // 2) + 4 * pp + ss % 4).ravel()
    for c in range(N_CORES):
        oc = res.results[c]["out"].reshape(P * 8, O)
        out[tok_lists[c][tl]] = oc.astype(np.float32)
    return out.reshape(B, S, O), res


def kernel(x, W, b, Wg, bg):
    out, _ = run({"x": x, "W": W, "b": b, "Wg": Wg, "bg": bg})
    return out



# revision 25
# speedup vs baseline: 1.0817x; 1.0817x over previous
"""MoE top-2 expert projection kernel for 8 Trainium2 NeuronCores — v4.

Expert-parallel-with-cover token dispatch (per the sharding hint): the host
computes the top-2 routing *decision* and uses it only to build the dispatch
layout.  Each core holds 4 of the 8 experts (an 8-block pair cover of K8), and
every token goes to a core owning both of its experts, so no cross-core
combine is needed.  All reference arithmetic — gate logits, softmax, expert
projection, bias, gate scaling, top-2 combine — runs on the NeuronCores:

  per slot-tile (128 dispatched slots, possibly spanning 2 experts):
    psum_g = xd^T Wg                      fp16 matmuls, fp32 psum
    psum_y = xd^T W_seg  (col-tiled: concurrent matmuls per expert subrange)
    g      = softmax(psum_g + bg) . onehot                (DVE + ACT exp)
    y16    = (psum_y + bias_bcast) * g                    (DVE add, ACT scale)
    plain store y16 -> yk[t*128 : t*128+ts]               (HWDGE DMA)
  pipelined combine: tokens sorted by max slot-tile, in 8 groups of 128;
  as soon as a group's last tile is stored, two indirect row-gathers pull
  its token pairs back, DVE sums them, and the result streams to out —
  overlapping the remaining tiles' compute.

Queue discipline (load-bearing): ALL input loads + stores + out stores issue
from the sync queue in need order (a dma_start stream blocks its sequencer
once queue credits run out, so bulk DMA on the scalar queue would stall the
ACT ops for ~35us); the scalar queue runs only ACT ops; gpsimd runs only the
8x2 gathers; readback order is enforced with per-group semaphores waited at
their full totals (sound under DMA completion sub-increment reordering).
"""

import sys

if "/opt/trn_rl_repo" not in sys.path:
    sys.path.insert(0, "/opt/trn_rl_repo")

import random

import numpy as np

import concourse.bass as bass
import concourse.mybir as mybir
import concourse.tile as tile
from concourse import bacc
from concourse.bass_utils import run_bass_kernel_spmd
from concourse.masks import make_identity

# Problem sizes (hardcoded per the harness contract).
B, S, D, O, E = 4, 2048, 1024, 1024, 8
TOK = B * S                  # 8192 tokens total
N_CORES = 8
TPC = TOK // N_CORES         # 1024 tokens per core
P = 128                      # SBUF partitions
KO = D // P                  # 8 contraction blocks
NH = O // 512                # PSUM halves of the output dim
EL = 4                       # experts resident per core
ALIGN = 64                   # segment-boundary alignment (PE col-group size)
NG = 8                       # readback/combine groups (128 tokens each)
NWARM = 14                   # PE warmup matmuls bridging the input-load head
# pair cover: 8 blocks of 4 experts, every expert in 4 blocks, all 28 pairs
BLOCKS = [(0, 1, 2, 3), (0, 1, 4, 5), (0, 1, 6, 7), (2, 3, 4, 5),
          (2, 3, 6, 7), (4, 5, 6, 7), (0, 2, 4, 6), (1, 3, 5, 7)]

F16 = mybir.dt.float16
F32 = mybir.dt.float32
I32 = mybir.dt.int32


def build_nc(S_seg, thr, ts_last):
    """S_seg: per-rank segment sizes (64-aligned), identical on all cores.
    thr: per readback group, the max slot-tile index any of its tokens
    uses (program-baked; the group's gathers fire once that tile's store
    has landed).  ts_last: row count of the final (partial) tile."""
    S_seg = tuple(int(s) for s in S_seg)
    thr = tuple(int(t) for t in thr)
    SLOTS = sum(S_seg)
    NT = -(-SLOTS // P)
    bnd = np.concatenate([[0], np.cumsum(S_seg)])

    # per tile: segment subranges [(col_off, len, rank)].  64-aligned
    # boundaries + segments >= 128 guarantee at most 2 segments per tile,
    # each a single legal PE column tile (64@0, 64@64 or 128@0).
    tile_seg = []
    for t in range(NT):
        t0, t1 = t * P, min((t + 1) * P, SLOTS)
        segs = []
        for l in range(EL):
            lo, hi = max(t0, bnd[l]), min(t1, bnd[l + 1])
            if lo < hi:
                segs.append((int(lo - t0), int(hi - lo), l))
        assert len(segs) <= 2
        for (soff, sln, _l) in segs:
            assert (soff, sln) in ((0, 128), (0, 64), (64, 64)), (t, segs)
        tile_seg.append(segs)

    nc = bacc.Bacc(None, target_bir_lowering=False)

    xd_in = nc.dram_tensor("xd", [P, NT * KO * P], F16, kind="ExternalInput")
    WT = nc.dram_tensor("WT", [EL, NH, P, KO * 512], F16, kind="ExternalInput")
    WgT = nc.dram_tensor("WgT", [P, KO * E], F16, kind="ExternalInput")
    bgb_in = nc.dram_tensor("bgb", [P, E], F32, kind="ExternalInput")
    bb_in = nc.dram_tensor("bb", [P, EL * O], F16, kind="ExternalInput")
    oh_in = nc.dram_tensor("oh", [P, NT * E], F32, kind="ExternalInput")
    gidx_in = nc.dram_tensor("gidx", [P, 2 * NG], I32, kind="ExternalInput")
    yk = nc.dram_tensor("yk", [NT * P, O], F16, kind="Internal")
    # out rows: out[p, g*O:(g+1)*O] = token with local id g*128+p
    out = nc.dram_tensor("out", [P, NG * O], F16, kind="ExternalOutput")

    rb_sems = [nc.alloc_semaphore(f"rb_sem{g}") for g in range(NG)]

    with tile.TileContext(nc) as tc:
        with (
            tc.tile_pool(name="resident", bufs=1) as rpool,
            tc.tile_pool(name="wres", bufs=1) as wrpool,
            tc.tile_pool(name="work", bufs=3) as wpool,
            tc.tile_pool(name="rb", bufs=2) as rbpool,
            tc.tile_pool(name="ob", bufs=2) as opool,
            tc.tile_pool(name="psum_y", bufs=2, space="PSUM") as ypool,
            tc.tile_pool(name="psum_y2", bufs=1, space="PSUM") as y2pool,
            tc.tile_pool(name="psum_g", bufs=2, space="PSUM") as gpool,
        ):
            xd16 = rpool.tile([P, NT, KO, P], F16, tag="xd16")
            wg16 = rpool.tile([P, KO, E], F16, tag="wg16")
            bgb32 = rpool.tile([P, E], F32, tag="bgb32")
            bb16 = rpool.tile([P, EL * O], F16, tag="bb16")
            oh32 = rpool.tile([P, NT * E], F32, tag="oh32")
            gidx32 = rpool.tile([P, 2 * NG], I32, tag="gidx32")
            ident = rpool.tile([P, P], F32, tag="ident")
            w_bufs = [
                wrpool.tile([P, KO, O], F16, tag=f"w{l}", name=f"wbuf{l}")
                for l in range(EL)
            ]

            # ---- initial loads (sync queue, need order) -----------------
            make_identity(nc, ident[:])
            nc.sync.dma_start(gidx32[:], gidx_in[:, :])
            nc.sync.dma_start(
                wg16[:], WgT[:, :].rearrange("p (ko e) -> p ko e", ko=KO)
            )
            nc.sync.dma_start(bgb32[:], bgb_in[:, :])
            nc.sync.dma_start(oh32[:], oh_in[:, :])

            def load_xd(t):
                nc.sync.dma_start(
                    xd16[:, t, :, :],
                    xd_in[:, t * KO * P:(t + 1) * KO * P].rearrange(
                        "p (ko s) -> p ko s", ko=KO
                    ),
                )

            def load_w(l, h):
                nc.sync.dma_start(
                    w_bufs[l][:, :, h * 512:(h + 1) * 512],
                    WT[l, h].rearrange("p (ko o) -> p ko o", ko=KO),
                )

            load_xd(0)
            load_xd(1)
            load_w(0, 0)
            load_w(0, 1)
            nc.sync.dma_start(bb16[:], bb_in[:, :])
            load_xd(2)
            load_xd(3)
            load_w(1, 0)
            load_w(1, 1)
            for t in range(4, min(8, NT)):
                load_xd(t)
            load_w(2, 0)
            load_w(2, 1)
            for t in range(8, min(12, NT)):
                load_xd(t)
            load_w(3, 0)
            load_w(3, 1)
            for t in range(12, NT):
                load_xd(t)

            # PE warmups: ramp HAM to K=8/8 and bridge the input-load head
            for i in range(NWARM):
                warm_ps = ypool.tile([P, O], F32, tag="y", name=f"warm{i}")
                nc.tensor.matmul(
                    warm_ps[:, 0:P], lhsT=ident[:], rhs=ident[:],
                    start=True, stop=True,
                )

            # readback groups: emitted during tile thr[g]+1, after its store.
            rb_at = {}
            for g in range(NG):
                rb_at.setdefault(min(thr[g] + 1, NT - 1), []).append(g)
            # store of tile t credits the first group needing it; group g's
            # rows are covered once sems 0..g reach their full totals
            G_of = [min(g for g in range(NG) if thr[g] >= t) if thr[-1] >= t
                    else None for t in range(NT)]
            cred = [sum(1 for t in range(NT) if G_of[t] == g)
                    for g in range(NG)]
            last_waited = [-1]

            def emit_group(g):
                # wait each newly-covered group sem at its FULL total: only
                # stores 0..thr[g'] increment rb_sems[g'], so the full total
                # is the unique sound wait under per-engine completion
                # sub-increment reordering.  Earlier sems were awaited by
                # earlier groups on this same (FIFO) gpsimd queue.
                for g2 in range(last_waited[0] + 1, g + 1):
                    if cred[g2]:
                        nc.gpsimd.wait_ge(rb_sems[g2], cred[g2] * 16)
                last_waited[0] = max(last_waited[0], g)
                rb = rbpool.tile([P, 2, O], F16, tag="rb", name=f"rb{g}")
                for r in range(2):
                    nc.gpsimd.indirect_dma_start(
                        out=rb[:, r, :],
                        out_offset=None,
                        in_=yk[:, :],
                        in_offset=bass.IndirectOffsetOnAxis(
                            ap=gidx32[:, 2 * g + r:2 * g + r + 1], axis=0
                        ),
                    )
                o16 = opool.tile([P, O], F16, tag="o16", name=f"o{g}")
                nc.vector.tensor_tensor(
                    o16[:], rb[:, 0, :], rb[:, 1, :], mybir.AluOpType.add
                )
                nc.sync.dma_start(out[:, g * O:(g + 1) * O], o16[:])

            # ---- slot tiles ---------------------------------------------
            for t in range(NT):
                ts_ = P if t < NT - 1 else ts_last
                sub = slice(0, ts_)
                segs = tile_seg[t]

                # gate logits for this tile
                psum_g = gpool.tile([P, E], F32, tag="g")
                for ko in range(KO):
                    nc.tensor.matmul(
                        psum_g[sub, :], lhsT=xd16[:, t, ko, sub],
                        rhs=wg16[:, ko, :],
                        start=(ko == 0), stop=(ko == KO - 1),
                    )
                # expert matmuls, h-major so each W half is consumed in
                # order.  Boundary tiles issue concurrent column-tiled
                # matmuls (the two experts run in disjoint PE column groups);
                # the second segment accumulates in its own PSUM tile so each
                # PSUM zero region hosts a single accumulation group.
                psum_y = ypool.tile([P, O], F32, tag="y")
                psums = [psum_y]
                if len(segs) > 1:
                    psums.append(
                        y2pool.tile([P, O], F32, tag="y2", name=f"py2_{t}")
                    )
                for h in range(NH):
                    hsl = slice(h * 512, (h + 1) * 512)
                    for ko in range(KO):
                        for si, (soff, sln, l) in enumerate(segs):
                            nc.tensor.matmul(
                                psums[si][soff:soff + sln, hsl],
                                lhsT=xd16[:, t, ko, soff:soff + sln],
                                rhs=w_bufs[l][:, ko, hsl],
                                start=(ko == 0), stop=(ko == KO - 1),
                                tile_position=(0, soff),
                            )

                # gate weight: g = softmax(logits)[slot's expert column]
                logits = wpool.tile([P, E], F32, tag="lg")
                nc.vector.tensor_tensor(
                    logits[sub, :], psum_g[sub, :], bgb32[sub, :],
                    mybir.AluOpType.add,
                )
                m1 = wpool.tile([P, 1], F32, tag="m1")
                nc.vector.tensor_reduce(
                    m1[sub, :], logits[sub, :], mybir.AxisListType.X,
                    mybir.AluOpType.max,
                )
                negm1 = wpool.tile([P, 1], F32, tag="nm")
                nc.vector.tensor_scalar_mul(negm1[sub, :], m1[sub, :], -1.0)
                ex = wpool.tile([P, E], F32, tag="ex")
                nc.scalar.activation(
                    ex[sub, :], logits[sub, :],
                    mybir.ActivationFunctionType.Exp,
                    bias=negm1[sub, 0:1], scale=1.0,
                )
                ssum = wpool.tile([P, 1], F32, tag="ss")
                nc.vector.tensor_reduce(
                    ssum[sub, :], ex[sub, :], mybir.AxisListType.X,
                    mybir.AluOpType.add,
                )
                rsum = wpool.tile([P, 1], F32, tag="rs")
                nc.vector.reciprocal(rsum[sub, :], ssum[sub, :])
                ohx = wpool.tile([P, E], F32, tag="ox")
                g0 = wpool.tile([P, 1], F32, tag="g0")
                nc.vector.tensor_tensor(
                    ohx[sub, :], ex[sub, :], oh32[sub, t * E:(t + 1) * E],
                    mybir.AluOpType.mult,
                )
                nc.vector.tensor_reduce(
                    g0[sub, :], ohx[sub, :], mybir.AxisListType.X,
                    mybir.AluOpType.add,
                )
                g_col = wpool.tile([P, 1], F32, tag="gc")
                nc.vector.tensor_tensor(
                    g_col[sub, :], g0[sub, :], rsum[sub, :],
                    mybir.AluOpType.mult,
                )

                # y16 = (psum_y + bias) * g ; plain store to yk slot rows
                y16a = wpool.tile([P, O], F16, tag="ya")
                for si, (soff, sln, l) in enumerate(segs):
                    ssl = slice(soff, soff + sln)
                    nc.vector.tensor_tensor(
                        y16a[ssl, :], psums[si][ssl, :],
                        bb16[ssl, l * O:(l + 1) * O], mybir.AluOpType.add,
                    )
                # distinct per-tile buffer: SWDGE-DMA reads aren't
                # dep-tracked, so pooled reuse would race with the store
                y16 = rpool.tile([P, O], F16, tag=f"yb{t}", name=f"y16_{t}")
                nc.scalar.activation(
                    y16[sub, :], y16a[sub, :],
                    mybir.ActivationFunctionType.Copy,
                    bias=0.0, scale=g_col[sub, 0:1],
                )
                # SWDGE store: carries the group-credit semaphore update
                # (HWDGE dma_start can't take an extra update), and keeps
                # stores+gathers on one FIFO queue
                st = nc.gpsimd.dma_start(yk[t * P:t * P + ts_, :], y16[sub, :])
                if G_of[t] is not None:
                    st.then_inc(rb_sems[G_of[t]], 16)

                # pipelined combine for groups unlocked by earlier tiles
                if t < NT - 1:
                    for g in rb_at.get(t, []):
                        emit_group(g)

            for g in rb_at.get(NT - 1, []):
                emit_group(g)

    nc.compile()
    return nc


_NC_CACHE = {}


def _get_nc(S_seg, thr, ts_last):
    key = (tuple(S_seg), tuple(thr), int(ts_last))
    if key not in _NC_CACHE:
        _NC_CACHE[key] = build_nc(*key)
    return _NC_CACHE[key]


def _route(x2d, Wg, bg):
    logits = x2d.astype(np.float32) @ Wg.astype(np.float32).T + bg
    m = logits.max(axis=1, keepdims=True)
    p = np.exp(logits - m, dtype=np.float32)
    p /= p.sum(axis=1, keepdims=True)
    order = np.argsort(-p, axis=1, kind="stable")
    return order[:, 0].astype(np.int64), order[:, 1].astype(np.int64)


def _balance(e1, e2):
    """Assign each token to a core whose 4-expert block contains both of
    its experts, minimizing the (64-aligned, rank-uniform) slot count.
    Deterministic: waterfill + seeded local search + core-count repair."""
    pairs = {}
    for t in range(TOK):
        k = (min(e1[t], e2[t]), max(e1[t], e2[t]))
        pairs.setdefault(k, []).append(t)
    cand = {k: [c for c in range(N_CORES)
                if k[0] in BLOCKS[c] and k[1] in BLOCKS[c]] for k in pairs}
    assert all(cand.values())
    CNT = np.bincount(np.concatenate([e1, e2]), minlength=E).astype(float)
    nblk = np.array([sum(1 for b in BLOCKS if e in b) for e in range(E)],
                    float)
    T = CNT / nblk
    f = np.zeros((N_CORES, E))
    quota = {k: {c: 0 for c in cand[k]} for k in pairs}
    porder = sorted(pairs, key=lambda k: (len(cand[k]), -len(pairs[k])))
    rem = {k: len(pairs[k]) for k in pairs}
    while any(rem.values()):
        for k in porder:
            if rem[k] == 0:
                continue
            n = min(8, rem[k])
            a, b = k
            c = min(cand[k], key=lambda c: (max(f[c, a] - T[a],
                                               f[c, b] - T[b]), f[c].sum()))
            quota[k][c] += n
            f[c, a] += n
            f[c, b] += n
            rem[k] -= n

    def J(f):
        al = np.ceil(np.sort(f, axis=1)[:, ::-1] / ALIGN) * ALIGN
        imb = np.abs(f.sum(axis=1) - 2 * TPC).sum()
        return al.max(axis=0).sum(), imb

    best = J(f)
    rng = random.Random(0)
    keys = [k for k in pairs if len(cand[k]) >= 2]
    for it in range(150000):
        k = keys[rng.randrange(len(keys))]
        c1, c2 = rng.sample(cand[k], 2)
        n = 1 if it % 3 else rng.choice([1, 2, 4, 8])
        if quota[k][c1] < n:
            continue
        a, b = k
        f[c1, a] -= n
        f[c1, b] -= n
        f[c2, a] += n
        f[c2, b] += n
        j2 = J(f)
        if j2 <= best:
            best = j2
            quota[k][c1] -= n
            quota[k][c2] += n
        else:
            f[c1, a] += n
            f[c1, b] += n
            f[c2, a] -= n
            f[c2, b] -= n
    # repair: force exactly TPC tokens per core (allow J to grow if needed)
    for it in range(20000):
        tot = f.sum(axis=1) / 2
        if (tot == TPC).all():
            break
        hi = int(np.argmax(tot))
        bestmv = None
        for k in keys:
            if hi not in cand[k] or quota[k][hi] == 0:
                continue
            for c2 in cand[k]:
                if tot[c2] >= TPC or c2 == hi:
                    continue
                a, b = k
                f[hi, a] -= 1
                f[hi, b] -= 1
                f[c2, a] += 1
                f[c2, b] += 1
                j2 = J(f)
                f[hi, a] += 1
                f[hi, b] += 1
                f[c2, a] -= 1
                f[c2, b] -= 1
                if bestmv is None or j2 < bestmv[0]:
                    bestmv = (j2, k, c2)
        assert bestmv is not None, "core-count repair stuck"
        _, k, c2 = bestmv
        a, b = k
        quota[k][hi] -= 1
        quota[k][c2] += 1
        f[hi, a] -= 1
        f[hi, b] -= 1
        f[c2, a] += 1
        f[c2, b] += 1
    assert (f.sum(axis=1) == 2 * TPC).all()
    # token -> core
    core = np.full(TOK, -1, np.int64)
    for k, q in quota.items():
        ts_ = pairs[k]
        i = 0
        for c in sorted(q):
            n = q[c]
            core[ts_[i:i + n]] = c
            i += n
        assert i == len(ts_)
    return core, f.astype(np.int64)


def _prepare(inputs):
    x = np.ascontiguousarray(
        np.asarray(inputs["x"], np.float32)).reshape(TOK, D)
    W = np.asarray(inputs["W"], np.float32)
    bb = np.asarray(inputs["b"], np.float32)
    Wg = np.asarray(inputs["Wg"], np.float32)
    bg = np.asarray(inputs["bg"], np.float32)

    e1, e2 = _route(x, Wg, bg)
    core, cnt = _balance(e1, e2)

    # per-core expert ranks (by count desc) and uniform segment sizes
    locs = [sorted(BLOCKS[c], key=lambda e: (-cnt[c, e], e))
            for c in range(N_CORES)]
    lcnt = np.array([[cnt[c, locs[c][l]] for l in range(EL)]
                     for c in range(N_CORES)])
    S_seg = tuple(int(ALIGN * -(-int(lcnt[:, l].max()) // ALIGN))
                  for l in range(EL))
    SLOTS = sum(S_seg)
    NT = -(-SLOTS // P)
    ts_last = SLOTS - P * (NT - 1)
    bnd = np.concatenate([[0], np.cumsum(S_seg)])

    in_maps = []
    tok_lists = []
    group_thr = np.zeros((N_CORES, NG), np.int64)
    for c in range(N_CORES):
        loc = locs[c]
        perm = loc + sorted(set(range(E)) - set(loc))
        toks = np.nonzero(core == c)[0]
        ntk = toks.size
        assert ntk == TPC and 2 * ntk <= SLOTS
        # slot placement: rank l tokens occupy segment l (pads at end).
        # two passes order each segment by the token's other-slot tile so
        # early readback groups unlock early.
        tk_rank = {}   # token -> (la, lb) local ranks, la = rank of e1
        lofc = {e: l for l, e in enumerate(loc)}
        seg_tokens = [[] for _ in range(EL)]
        for tok in toks:
            la, lb = lofc[int(e1[tok])], lofc[int(e2[tok])]
            tk_rank[tok] = (la, lb)
            seg_tokens[la].append(tok)
            seg_tokens[lb].append(tok)
        pos = {}   # (token, rank) -> global slot
        for _pass in range(2):
            for l in range(EL):
                if _pass:
                    seg_tokens[l].sort(
                        key=lambda tok: (
                            pos[(tok, tk_rank[tok][1]
                                 if tk_rank[tok][0] == l
                                 else tk_rank[tok][0])] // P, tok))
                for i, tok in enumerate(seg_tokens[l]):
                    pos[(tok, l)] = int(bnd[l]) + i
        # local ids by max slot-tile
        mx = np.array([max(pos[(tok, tk_rank[tok][0])],
                           pos[(tok, tk_rank[tok][1])]) // P
                       for tok in toks])
        o_ = np.argsort(mx, kind="stable")
        toks = toks[o_]
        mx = mx[o_]
        for g in range(NG):
            group_thr[c, g] = mx[g * P:(g + 1) * P].max()

        # build xd slots, onehot, gather indices
        xd = np.zeros((SLOTS, D), np.float16)
        oh = np.zeros((SLOTS, E), np.float32)
        for l in range(EL):
            for tok in seg_tokens[l]:
                s = pos[(tok, l)]
                xd[s] = x[tok].astype(np.float16)
                oh[s, l] = 1.0
        gidx = np.empty((P, 2 * NG), np.int32)
        for g in range(NG):
            for p in range(P):
                tok = int(toks[g * P + p])
                la, lb = tk_rank[tok]
                s1, s2 = pos[(tok, la)], pos[(tok, lb)]
                gidx[p, 2 * g] = min(s1, s2)
                gidx[p, 2 * g + 1] = max(s1, s2)

        im = {}
        # xd: [P, NT*KO*P], tile-major 2KB lines
        xdp = np.zeros((NT * P, D), np.float16)
        xdp[:SLOTS] = xd
        im["xd"] = np.ascontiguousarray(
            xdp.T.reshape(KO, P, NT, P).transpose(1, 2, 0, 3)
            .reshape(P, NT * KO * P)
        )
        # W: [EL, NH, P, KO*512]
        wt = np.empty((EL, NH, P, KO * 512), np.float16)
        for l in range(EL):
            wT = W[loc[l]].T.astype(np.float16)   # [D, O]
            for h in range(NH):
                wt[l, h] = (wT[:, h * 512:(h + 1) * 512]
                            .reshape(KO, P, 512).transpose(1, 0, 2)
                            .reshape(P, KO * 512))
        im["WT"] = np.ascontiguousarray(wt)
        im["WgT"] = np.ascontiguousarray(
            Wg.T[:, perm].astype(np.float16)
            .reshape(KO, P, E).transpose(1, 0, 2).reshape(P, KO * E)
        )
        im["bgb"] = np.ascontiguousarray(
            np.broadcast_to(bg[perm].astype(np.float32), (P, E)))
        im["bb"] = np.ascontiguousarray(np.broadcast_to(
            bb[loc].astype(np.float16).reshape(1, EL * O), (P, EL * O)))
        ohp = np.zeros((NT * P, E), np.float32)
        ohp[:SLOTS] = oh
        im["oh"] = np.ascontiguousarray(
            ohp.reshape(NT, P, E).transpose(1, 0, 2).reshape(P, NT * E))
        im["gidx"] = np.ascontiguousarray(gidx)
        in_maps.append(im)
        tok_lists.append(toks)

    thr = tuple(int(group_thr[:, g].max()) for g in range(NG))
    return S_seg, thr, ts_last, in_maps, tok_lists


def run(inputs, **spmd_kwargs):
    S_seg, thr, ts_last, in_maps, tok_lists = _prepare(inputs)
    nc = _get_nc(S_seg, thr, ts_last)
    res = None
    for attempt in range(3):
        try:
            res = run_bass_kernel_spmd(
                nc, in_maps, core_ids=list(range(N_CORES)), **spmd_kwargs
            )
            break
        except Exception:
            if attempt == 2:
                raise
    out = np.empty((TOK, O), np.float32)
    for c in range(N_CORES):
        oc = res.results[c]["out"]                     # [P, NG*O]
        loc_out = (oc.reshape(P, NG, O).transpose(1, 0, 2)
                   .reshape(NG * P, O)).astype(np.float32)
        out[tok_lists[c]] = loc_out[:tok_lists[c].size]
    return out.reshape(B, S, O), res


def kernel(x, W, b, Wg, bg):
    out, _ = run({"x": x, "W": W, "b": b, "Wg": Wg, "bg": bg})
    return out
